# revision 19
# baseline (speedup 1.0000x reference)
"""Trainium2 Bass kernel for the 4-layer soft-logic-gate cellular automaton.

Hardcoded for x:(32,128,128) f32, toggle_gates:(4,16,256,256) f32, 8 cores.

Sharding: spatial over state rows H=256 with a redundant halo -> zero
cross-core communication. Core i consumes x rows [16i,16i+18) (mod 128),
gate rows [32i,32i+34) (mod 256), produces out rows [16i,16i+16).

Math: per pixel out = sum_c sigmoid(g_c) * prod_n(bit_n(c)? v_n : 1-v_n)
over the 2x2 torus neighborhood. Evaluated in the multilinear (Moebius)
basis: out = sum_{q,r} A_qr * n_q(row h) * n_r(row h+1),
n = [1, vR, vL, vL*vR]; A = 16 batch-free coefficient maps per layer
(on-device: ACT sigmoid -> 4 in-place DVE subtracts per layer).

Layout: states split by column parity into SE/SO tiles
[128 partitions = w', free = (h, b)], fp16 so tensor_tensor hits the DVE
2x_1P perf mode. w+1 for odd columns needs partition+1: materialized once
per layer via SBUF->SBUF DMA (engines cannot read across partitions).
Coefficients are x2-replicated innermost ([...,h,2]) so every operand has
a 16-bit step-1 4-byte-aligned innermost run, which keeps every
tensor_tensor in the DVE 2x_1P perf mode (verified on HW via a
loop-amplified microbench). The ISA allows max 3 free dims per operand;
the inner-stage ops fuse all four r-channels by spanning all HS rows so
the (r, h) AP dims coalesce at lowering. Layer 0 collapses (3 of 4
neighbors zero); layer 3 computes only even rows of even columns. The
reference clip(0,1) is a mathematical no-op (the truth-table weights are
a partition of unity, so outputs stay in [0,1]) and is dropped.
"""

import sys

sys.path.insert(0, "/opt/trn_rl_repo")

import numpy as np

import concourse.bacc as bacc
import concourse.mybir as mybir
from concourse.bass_utils import run_bass_kernel_spmd
from concourse.tile import TileContext

P = 128          # partitions = w' (column pairs)
B = 32           # batch
HS = 34          # local state rows = gate rows (33 needed + 1; state_1 has 34)
HG = 34          # gate-row storage
HX = 18          # local x rows
NCORES = 8
F32 = mybir.dt.float32
F16 = mybir.dt.float16

# Batch columns mirrored onto the Pool (gpsimd) engine. The whole Horner
# network decomposes over batch, so Pool runs an independent copy of every
# layer-op on b[BD:32] while DVE handles b[0:BD] - no intra-layer
# cross-engine deps (states are read b-sliced; coefficients are shared
# read-only). Pool tensor ops cost ~1.98ns/elem vs DVE's 0.52, so ~6/32
# of the batch balances the two engines once DVE also carries the Moebius.
BP = 6           # must be even (the (c j) fp16-pair views split at c = b//2)
BD = B - BP

_NC_CACHE = {}


def _build():
    nc = bacc.Bacc("TRN2", target_bir_lowering=False, debug=False, num_devices=NCORES)
    x_in = nc.dram_tensor("x", [P, HX, B], F32, kind="ExternalInput")
    g_in = nc.dram_tensor("g", [P, 2, 4, 4, 4, HG], F32, kind="ExternalInput")
    o_out = nc.dram_tensor("o", [P, 16, B], F16, kind="ExternalOutput")

    with TileContext(nc) as tc:
        with tc.tile_pool(name="pool", bufs=1) as pool:
            # ACT warmup: touch Sigmoid and Copy so their LUT function-set
            # loads (~1.3us each) overlap the first gate DMA instead of
            # serializing after it.
            warm = pool.tile([P, 2], F16, tag="warm")
            nc.vector.memset(warm[:], 0.0)
            nc.scalar.activation(
                warm[:, 0:1], warm[:, 0:1], mybir.ActivationFunctionType.Sigmoid
            )
            nc.scalar.copy(warm[:, 1:2], warm[:, 0:1])

            # ---- gate pipeline: DMA -> sigmoid(fp16) -> Moebius -> x2-replicate
            gt = pool.tile([P, 2, 4, 4, 4, HG], F32, tag="gt")
            sg = pool.tile([P, 2, 4, 4, 4, HG], F16, tag="sg")
            rep = pool.tile([P, 2, 4, 4, 4, HG, 2], F16, tag="rep")

            def prep(l, ews=(slice(0, 2),), split=False):
                # l may be an int or a layer slice (power-of-2 aligned slices
                # keep the Moebius APs within 3 free dims after coalescing).
                # With split=True (requires a single-layer l slice so the
                # partial-q/r APs still coalesce to <=3 free dims) the Moebius
                # butterflies are split 3:1 between DVE and Pool: passes a3/a2
                # are per-q independent (slice q), passes a1/a0 mix q (slice r).
                for ewsl in ews:
                    nc.sync.dma_start(out=gt[:, ewsl, l], in_=g_in[:, ewsl, l])
                    s = sg[:, ewsl, l]
                    nc.scalar.activation(
                        s, gt[:, ewsl, l], mybir.ActivationFunctionType.Sigmoid
                    )
                    engs32 = engs = (
                        ((nc.vector, slice(0, 3)), (nc.gpsimd, slice(3, 4)))
                        if split
                        else ((nc.vector, slice(0, 4)),)
                    )
                    for eng, sl in engs32:  # a3: odd r -= even r ; a2: r{2,3} -= r{0,1}
                        eng.tensor_sub(
                            s[:, :, :, sl, 1::2, :], s[:, :, :, sl, 1::2, :], s[:, :, :, sl, 0::2, :]
                        )
                        eng.tensor_sub(
                            s[:, :, :, sl, 2:4, :], s[:, :, :, sl, 2:4, :], s[:, :, :, sl, 0:2, :]
                        )
                    for eng, sl in engs:  # a1: odd q -= even q ; a0: q{2,3} -= q{0,1}
                        eng.tensor_sub(
                            s[:, :, :, 1::2, sl, :], s[:, :, :, 1::2, sl, :], s[:, :, :, 0::2, sl, :]
                        )
                        eng.tensor_sub(
                            s[:, :, :, 2:4, sl, :], s[:, :, :, 2:4, sl, :], s[:, :, :, 0:2, sl, :]
                        )
                    nc.scalar.copy(
                        rep[:, ewsl, l],
                        s.unsqueeze(6).broadcast_to(list(s.shape) + [2]),
                    )

            def cof(ew, l, q, r, rows=slice(0, HS)):
                # coeff view shaped [P, n, 16, 2] (b split so innermost is
                # a step-1 pair; the (0,16) broadcast sits in the middle)
                s = rep[:, ew, l, q, r, rows, :]          # [P, n, 2]
                n = s.shape[1]
                return s.unsqueeze(2).broadcast_to([P, n, B // 2, 2])

            def st2(ap):
                # [P, n, B] state/temp view -> [P, n, B//2, 2]
                return ap.rearrange("p h (c j) -> p h c j", j=2)

            # Horner coefficients for layers 1-2: a3-split, 8 leaf channels
            # contiguous, h-extent 33 so (channel, h) coalesces in oct ops.
            rep2 = pool.tile([P, 2, 2, 2, 8, 33, 2], F16, tag="rep2")

            def rep2_fill(li, l):
                for ew in (0, 1):
                    for a3 in (0, 1):
                        src = sg[:, ew, l, :, a3::2, 0:33].rearrange(
                            "p q r h -> p (q r) h"
                        )
                        nc.scalar.copy(
                            rep2[:, ew, li, a3],
                            src.unsqueeze(3).broadcast_to([P, 8, 33, 2]),
                        )

            def c8(ew, li, a3):
                sq = rep2[:, ew, li, a3]                  # [P, 8, 33, 2]
                return sq.unsqueeze(3).broadcast_to([P, 8, 33, B // 2, 2])

            def b8(ap):
                # [P, 33, B] -> broadcast [P, 8, 33, B//2, 2]
                return (
                    ap.rearrange("p h (c j) -> p h c j", j=2)
                    .unsqueeze(1)
                    .broadcast_to([P, 8, 33, B // 2, 2])
                )

            def cofq(ew, l, q):
                # all-r all-rows coeff view [P, 4, HS, 16, 2]; the (r, h)
                # dims coalesce at lowering into one dim, so the ISA sees
                # only 3 free dims.
                sq = rep[:, ew, l, q, :, :, :]            # [P, 4, HS, 2]
                return sq.unsqueeze(3).broadcast_to([P, 4, HS, B // 2, 2])

            def stq(ap):
                # full-rows [P, HS, B] state view -> [P, 4, HS, B//2, 2]
                return (
                    ap.rearrange("p h (c j) -> p h c j", j=2)
                    .unsqueeze(1)
                    .broadcast_to([P, 4, HS, B // 2, 2])
                )

            # ---- x load (f32 DMA + ACT cast; keeps Pool free for compute),
            # shifted copy; gates first so the SP queue never stalls behind
            # the xt-dependent shift DMAs
            xt = pool.tile([P, HX, B], F16, tag="xt")
            xtf = pool.tile([P, HX, B], F32, tag="xtf")
            nc.sync.dma_start(out=xtf[:], in_=x_in[:])
            nc.scalar.copy(xt[:], xtf[:])

            prep(slice(0, 1), ews=(slice(0, 1), slice(1, 2)))

            prep(slice(1, 2), split=True)
            rep2_fill(0, 1)

            xs = pool.tile([P, HX, B], F16, tag="xs")
            nc.sync.dma_start(out=xs[0:127], in_=xt[1:128])
            nc.sync.dma_start(out=xs[127:128], in_=xt[0:1])

            # ---- layer 0: state_1 (rows 0..33) from zero-upsampled x ----
            SE = pool.tile([P, HS, B], F16, tag="se1")
            SO = pool.tile([P, HS, B], F16, tag="so1")
            ev_c, od_c = slice(0, 33, 2), slice(1, 34, 2)  # 17 rows each
            for (ew, outt, xsrc, xr, qa, ra, rows_o, rows_c) in (
                (0, SE, xt, slice(0, 17), 2, 0, slice(0, 34, 2), ev_c),
                (0, SE, xt, slice(1, 18), 0, 2, slice(1, 34, 2), od_c),
                (1, SO, xs, slice(0, 17), 1, 0, slice(0, 34, 2), ev_c),
                (1, SO, xs, slice(1, 18), 0, 1, slice(1, 34, 2), od_c),
            ):
                t0 = pool.tile([P, 17, B], F16, tag="l0t", bufs=2)
                nc.vector.tensor_mul(
                    st2(t0[:]), st2(xsrc[:, xr, :]), cof(ew, 0, qa, ra, rows_c)
                )
                nc.vector.tensor_add(
                    st2(outt[:, rows_o, :]), st2(t0[:]), cof(ew, 0, 0, 0, rows_c)
                )

            def inner(l, ew, nR, nL, u):
                """Fused-over-r inner sums: S[r] = A0r + A1r*nR + A2r*nL + A3r*u.
                Quad ops over all r and all HS rows (APs coalesce to 3 dims);
                garbage tail rows are finite and never harvested."""
                T1 = pool.tile([P, 4, HS, B], F16, tag="T1", bufs=2)
                T2 = pool.tile([P, 4, HS, B], F16, tag="T2", bufs=2)
                T3 = pool.tile([P, 4, HS, B], F16, tag="T3", bufs=2)
                q1 = T1[:].rearrange("p r h (c j) -> p r h c j", j=2)
                q2 = T2[:].rearrange("p r h (c j) -> p r h c j", j=2)
                q3 = T3[:].rearrange("p r h (c j) -> p r h c j", j=2)
                nc.vector.tensor_mul(q1, stq(nR), cofq(ew, l, 1))
                nc.vector.tensor_mul(q2, stq(nL), cofq(ew, l, 2))
                nc.vector.tensor_mul(q3, stq(u), cofq(ew, l, 3))
                nc.vector.tensor_add(q1, q1, cofq(ew, l, 0))
                nc.vector.tensor_add(T2[:], T2[:], T3[:])
                nc.vector.tensor_add(T1[:], T1[:], T2[:])
                return T1

            def outer(S, nRb, nLb, ub, o, n, ew=0):
                """out = (S0 + S1*nR') + (S2*nL' + S3*u') as a balanced tree."""
                ta = pool.tile([P, HS, B], F16, tag="ta", bufs=2)
                tb = pool.tile([P, HS, B], F16, tag="tb", bufs=2)
                nc.vector.tensor_mul(ta[:, 0:n, :], S[:, 1, 0:n, :], nRb)
                nc.vector.tensor_mul(tb[:, 0:n, :], S[:, 2, 0:n, :], nLb)
                nc.vector.tensor_mul(o, S[:, 3, 0:n, :], ub)
                nc.vector.tensor_add(ta[:, 0:n, :], ta[:, 0:n, :], S[:, 0, 0:n, :])
                nc.vector.tensor_add(o, o, tb[:, 0:n, :])
                nc.vector.tensor_add(o, o, ta[:, 0:n, :])

            # ---- layers 1, 2: 4-level Horner (f = even + v*odd per bit) ----
            for l in (1, 2):
                if l == 2:
                    prep(slice(2, 3), split=True)
                    prep(slice(3, 4), split=True)
                    rep2_fill(1, 2)
                li = l - 1
                n = 34 - l          # valid output rows
                SESH = pool.tile([P, HS, B], F16, tag="sesh", bufs=2)
                nc.sync.dma_start(out=SESH[0:127], in_=SE[1:128])
                nc.sync.dma_start(out=SESH[127:128], in_=SE[0:1])
                SEn = pool.tile([P, HS, B], F16, tag=f"se{l + 1}")
                SOn = pool.tile([P, HS, B], F16, tag=f"so{l + 1}")
                nc.scalar.memzero(SEn[:, 33:HS, :])
                nc.scalar.memzero(SOn[:, 33:HS, :])
                rows = slice(0, 33)
                below = slice(1, 34)
                for ew, (v0m, v1m, outt) in enumerate(
                    [(SE, SO, SEn), (SO, SESH, SOn)]
                ):
                    T8 = pool.tile([P, 8, HS, B], F16, tag="T8", bufs=2)
                    T4 = pool.tile([P, 4, HS, B], F16, tag="T4", bufs=2)
                    T2d = pool.tile([P, 2, HS, B], F16, tag="T2d", bufs=2)
                    # leaf level + all muls run b-split on DVE/Pool; the
                    # even+odd combine adds run as single full-B SBUF->SBUF
                    # CCE DMAs (out += in) - ~1.1us of Pool SWDGE each vs
                    # 2-4us of engine ALU time.
                    for eng, b0, b1 in ((nc.vector, 0, BD), (nc.gpsimd, BD, B)):
                        nb = (b1 - b0) // 2
                        q8 = T8[:, :, rows, b0:b1].rearrange(
                            "p r h (c j) -> p r h c j", j=2
                        )
                        v3b = (
                            v1m[:, below, b0:b1]
                            .rearrange("p h (c j) -> p h c j", j=2)
                            .unsqueeze(1)
                            .broadcast_to([P, 8, 33, nb, 2])
                        )
                        c81 = rep2[:, ew, li, 1].unsqueeze(3).broadcast_to(
                            [P, 8, 33, nb, 2]
                        )
                        c80 = rep2[:, ew, li, 0].unsqueeze(3).broadcast_to(
                            [P, 8, 33, nb, 2]
                        )
                        # level 3: leaf_k = C[k,a3=0] + v3*C[k,a3=1], v3=v1m@h+1
                        eng.tensor_mul(q8, v3b, c81)
                        eng.tensor_add(q8, q8, c80)
                        # level 2 mul: v2*odd, v2 = v0m@h+1
                        v2b = (
                            v0m[:, below, b0:b1]
                            .unsqueeze(1)
                            .broadcast_to([P, 4, 33, b1 - b0])
                        )
                        eng.tensor_mul(
                            T4[:, :, rows, b0:b1], v2b, T8[:, 1::2, rows, b0:b1]
                        )
                        eng.tensor_add(
                            T4[:, :, rows, b0:b1],
                            T4[:, :, rows, b0:b1],
                            T8[:, 0::2, rows, b0:b1],
                        )
                        # level 1: v1 = v1m@h
                        v1b = (
                            v1m[:, rows, b0:b1]
                            .unsqueeze(1)
                            .broadcast_to([P, 2, 33, b1 - b0])
                        )
                        eng.tensor_mul(
                            T2d[:, :, rows, b0:b1], v1b, T4[:, 1::2, rows, b0:b1]
                        )
                        eng.tensor_add(
                            T2d[:, :, rows, b0:b1],
                            T2d[:, :, rows, b0:b1],
                            T4[:, 0::2, rows, b0:b1],
                        )
                        # level 0: out = even + v0*odd, v0 = v0m@h
                        o = outt[:, rows, b0:b1]
                        eng.tensor_mul(o, v0m[:, rows, b0:b1], T2d[:, 1, rows, b0:b1])
                        eng.tensor_add(o, o, T2d[:, 0, rows, b0:b1])
                SE, SO = SEn, SOn

            # ---- layer 3: 4-level Horner on even rows of even cols ----
            rep3 = pool.tile([P, 2, 8, 16, 2], F16, tag="rep3")
            for a3 in (0, 1):
                src = sg[:, 0, 3, :, a3::2, 0:31:2].rearrange("p q r h -> p (q r) h")
                nc.scalar.copy(
                    rep3[:, a3], src.unsqueeze(3).broadcast_to([P, 8, 16, 2])
                )

            def c83(a3):
                return rep3[:, a3].unsqueeze(3).broadcast_to([P, 8, 16, B // 2, 2])

            ev = slice(0, 32, 2)   # 16 rows 0,2,...,30
            od = slice(1, 33, 2)   # 1,3,...,31
            T8L = pool.tile([P, 8, 16, B], F16, tag="T8L")
            T4L = pool.tile([P, 4, 16, B], F16, tag="T4L")
            T2L = pool.tile([P, 2, 16, B], F16, tag="T2L")
            out_t = pool.tile([P, 16, B], F16, tag="out")
            for eng, b0, b1 in ((nc.vector, 0, BD), (nc.gpsimd, BD, B)):
                nb = (b1 - b0) // 2
                q8L = T8L[:, :, :, b0:b1].rearrange("p r h (c j) -> p r h c j", j=2)
                v3b = (
                    SO[:, od, b0:b1].rearrange("p h (c j) -> p h c j", j=2)
                    .unsqueeze(1)
                    .broadcast_to([P, 8, 16, nb, 2])
                )
                c83b1 = rep3[:, 1].unsqueeze(3).broadcast_to([P, 8, 16, nb, 2])
                c83b0 = rep3[:, 0].unsqueeze(3).broadcast_to([P, 8, 16, nb, 2])
                eng.tensor_mul(q8L, v3b, c83b1)
                eng.tensor_add(q8L, q8L, c83b0)
                v2b = SE[:, od, b0:b1].unsqueeze(1).broadcast_to([P, 4, 16, b1 - b0])
                eng.tensor_mul(T4L[:, :, :, b0:b1], v2b, T8L[:, 1::2, :, b0:b1])
                eng.tensor_add(
                    T4L[:, :, :, b0:b1], T4L[:, :, :, b0:b1], T8L[:, 0::2, :, b0:b1]
                )
                v1b = SO[:, ev, b0:b1].unsqueeze(1).broadcast_to([P, 2, 16, b1 - b0])
                eng.tensor_mul(T2L[:, :, :, b0:b1], v1b, T4L[:, 1::2, :, b0:b1])
                eng.tensor_add(
                    T2L[:, :, :, b0:b1], T2L[:, :, :, b0:b1], T4L[:, 0::2, :, b0:b1]
                )
                eng.tensor_mul(
                    out_t[:, :, b0:b1], SE[:, ev, b0:b1], T2L[:, 1, :, b0:b1]
                )
                eng.tensor_add(
                    out_t[:, :, b0:b1], out_t[:, :, b0:b1], T2L[:, 0, :, b0:b1]
                )
            nc.sync.dma_start(out=o_out[:], in_=out_t[:])

    nc.compile()
    return nc


def _get_nc():
    if "nc" not in _NC_CACHE:
        _NC_CACHE["nc"] = _build()
    return _NC_CACHE["nc"]


def _shard_inputs(x, toggle_gates):
    in_maps = []
    for i in range(NCORES):
        xrows = np.arange(16 * i, 16 * i + HX) % 128
        xs = np.ascontiguousarray(x[:, xrows, :].transpose(2, 1, 0))  # (w,h',b)
        grow = np.arange(32 * i, 32 * i + HG) % 256
        g = toggle_gates[:, :, grow, :].transpose(3, 0, 1, 2)  # (w,l,c,h)
        g = np.ascontiguousarray(g).reshape(P, 2, 4, 4, 4, HG).astype(np.float32)
        in_maps.append({"x": xs, "g": g})
    return in_maps


def kernel(x, toggle_gates):
    x = np.asarray(x, dtype=np.float32)
    toggle_gates = np.asarray(toggle_gates, dtype=np.float32)
    nc = _get_nc()
    in_maps = _shard_inputs(x, toggle_gates)
    res = run_bass_kernel_spmd(nc, in_maps, list(range(NCORES)))
    out = np.empty((B, 128, 128), np.float32)
    for i in range(NCORES):
        o = res.results[i]["o"].astype(np.float32)  # (128 w', 16 y, 32 b)
        out[:, 16 * i : 16 * i + 16, :] = o.transpose(2, 1, 0)
    return out

